# revision 47
# baseline (speedup 1.0000x reference)
"""Trainium2 Bass kernel for DeBERTa-style disentangled self-attention
(nn_BertAttention_609885357022).

Sharding: 8 cores = 4 batches x 2 head-groups. Core c handles batch c//2,
heads [8*(c%2), 8*(c%2)+8). The two cores of a batch pair ReduceScatter their
partial output projections; core 2b keeps tokens [0:512), core 2b+1 keeps
tokens [512:1024). Host reassembles the full [4, 1024, 1024] output.

Changes over the staged baseline:
- fp8 (e4m3) operands end-to-end on the attention path: in_proj/pos-proj
  weights and activations, the banded c2p/p2c DRAM tiles, the gather tiles,
  probs (e1) and ctx. PSUM accumulation stays fp32; the residual+LN path
  stays fp32. Halves HBM traffic and SBUF footprint; rel-err ~2e-3.
- S3/S2/v/produce are interleaved per channel-chunk and produce blocks are
  threaded through consume's J-loop, so band writes/gathers start ~20us in
  and the PSUM-evacuation copies overlap consume's PE work.
- ctx/Z rows are staged out of PSUM (craw) immediately after the PV matmul
  so the 1/Z DRAM-broadcast round trip stays off the PSUM-recycle path.
- Band writes issue on the Act HWDGE ring, gathers on the SP ring; band
  PSUM->SBUF copies alternate DVE/Act per chunk to balance both engines.
- Scale folding: wq/qb and wpq/pqb are pre-divided by sqrt(3*D) host-side.

Score layout is S^T ([key j partitions, query i free]); probs feed the PV
matmul directly as the moving operand with V (+ones columns for Z) as the
stationary. The two relative-position terms share ONE banded DRAM tile per
head ([S, 2*BAND]: c2p reversed | p2c raw); a single diagonal-gather DMA per
128-block shears both at once (row stride 2*BAND-1 against a 2*BAND-pitch
layout). g[I][p, j] = c2p_att[i=128I+p, i-j+512] (S layout -> PE-transposed
onto the qk PSUM); g[J][p, S+i] = p2c_att[j=128J+p, i-j+512] (S^T layout ->
DVE add). No softmax max-subtraction is needed (|scores| < 4).
"""

import math
import os
import sys

for p in ("/opt/trn_rl_repo",):
    if os.path.isdir(p) and p not in sys.path:
        sys.path.insert(0, p)

import numpy as np
import ml_dtypes

import concourse.bass as bass
import concourse.bacc as bacc
import concourse.tile as tile
import concourse.mybir as mybir
from concourse.masks import make_identity

S = 1024
HID = 1024
D = 64
NB = 8
BAND = 1152
W2 = 2048
SCALE = math.sqrt(D * 3)
LN_EPS = 1e-7
FP = mybir.dt.float32
BF = mybir.dt.bfloat16
F8 = mybir.dt.float8e4
EXPF = mybir.ActivationFunctionType.Exp
COPYF = mybir.ActivationFunctionType.Copy
SQRTF = mybir.ActivationFunctionType.Sqrt


def build_kernel(sim_single_core=False, sim_rank=0, repeat=1, mask=0x1FF):
    nc = bacc.Bacc("TRN2", target_bir_lowering=False, debug=False, num_devices=8)

    din = {}
    for name, shape, dt in [
        ("hT", [128, 8 * S], F8),       # h^T: [c-part, kt*1024 + t]
        ("hres", [512, HID], FP),       # hidden[b, my half] + out_b (fp32)
        ("wq", [128, 8 * 512], F8),     # [k-part, kt*512 + c]; pre-scaled 1/SCALE
        ("wk", [128, 8 * 512], F8),
        ("wv", [128, 8 * 512], F8),
        ("wpk", [128, 8 * 512], F8),
        ("wpq", [128, 8 * 512], F8),    # pre-scaled 1/SCALE
        ("relT", [128, 8 * S], F8),     # [k-part, kt*1024 + u]
        ("wo", [128, 4 * HID], F8),     # [cin-part, ci*1024 + cout]
        ("qb", [512], FP),              # pre-scaled 1/SCALE
        ("pqb", [512], FP),             # pre-scaled 1/SCALE
        ("vb", [512], FP),
        ("lng", [HID], FP),
        ("lnb", [HID], FP),
    ]:
        din[name] = nc.declare_dram_parameter(name, shape, dt, isOutput=False)
    dout = nc.declare_dram_parameter("out", [512, HID], FP, isOutput=True)

    import contextlib
    with tile.TileContext(nc) as tc, contextlib.ExitStack() as cctx:
        # Constants live across all repeats: avoids the cross-body WAR chain
        # (body n+1's setup waiting on body n's LN reads of lnb_rep) that
        # otherwise serializes consecutive iterations, and drops ~1.2MB of
        # redundant per-iteration broadcast loads.
        const = cctx.enter_context(tc.tile_pool(name="const", bufs=1))
        id_f = const.tile([128, 128], FP)
        make_identity(nc, id_f[:])
        id8 = const.tile([128, 128], F8)
        nc.vector.tensor_copy(id8[:], id_f[:])
        qb_sb = const.tile([128, 4], FP)   # qb_sb[p, ct] = qb[128*ct + p]
        pqb_sb = const.tile([128, 4], FP)
        vb_rep = const.tile([128, 512], FP)
        lng_rep = const.tile([128, HID], FP)
        lnb_rep = const.tile([128, HID], FP)
        eps_sb = const.tile([128, 1], FP)
        nc.vector.memset(eps_sb[:], LN_EPS)
        nc.sync.dma_start(qb_sb[:], bass.AP(din["qb"], 0, [[1, 128], [128, 4]]))
        nc.sync.dma_start(pqb_sb[:], bass.AP(din["pqb"], 0, [[1, 128], [128, 4]]))
        nc.sync.dma_start(vb_rep[:], bass.AP(din["vb"], 0, [[0, 128], [1, 512]]))
        nc.sync.dma_start(lng_rep[:], bass.AP(din["lng"], 0, [[0, 128], [1, HID]]))
        nc.sync.dma_start(lnb_rep[:], bass.AP(din["lnb"], 0, [[0, 128], [1, HID]]))
        consts = dict(id8=id8, qb_sb=qb_sb, pqb_sb=pqb_sb, vb_rep=vb_rep,
                      lng_rep=lng_rep, lnb_rep=lnb_rep, eps_sb=eps_sb)
        for _ in range(repeat):
            _body(nc, tc, din, dout, sim_single_core, sim_rank, mask, consts)
    nc.compile()
    return nc


def _body(nc, tc, din, dout, sim_single_core, sim_rank, mask, consts):
    import contextlib
    id8 = consts["id8"]
    qb_sb = consts["qb_sb"]
    pqb_sb = consts["pqb_sb"]
    vb_rep = consts["vb_rep"]
    lng_rep = consts["lng_rep"]
    lnb_rep = consts["lnb_rep"]
    eps_sb = consts["eps_sb"]
    ctx = contextlib.ExitStack()
    with ctx:
        persist = ctx.enter_context(tc.tile_pool(name="persist", bufs=1))
        dram = ctx.enter_context(tc.tile_pool(name="dram", bufs=4, space="DRAM"))
        dram1 = ctx.enter_context(tc.tile_pool(name="dram1", bufs=1, space="DRAM"))

        # ---- persistent activations ----
        qT = persist.tile([128, 4 * S], F8)      # [c-part, ct*1024 + t]
        kT = persist.tile([128, 4 * S], F8)
        vaug = persist.tile([128, 8 * 1024], F8)  # [t-part, tt*1024 + 256*hh + ...]
        pkext = persist.tile([128, 4 * W2], F8)  # [c-part, ct*2048 + m]
        wo_sb = persist.tile([128, 4 * HID], F8)  # [cin-part, ci*1024 + cout]
        pqext = persist.tile([128, 4 * W2], F8)
        ctxT = persist.tile([128, 4 * S], F8)    # [c-part, ct*1024 + t]

        s1pool = ctx.enter_context(tc.tile_pool(name="s1", bufs=1))
        # PSUM budget (8 banks): ps_mm 2x[128,512] = 2, ps_s 2x[128,1024] = 4,
        # ps_ctx 1x[128,1024] = 2.
        ps_mm = ctx.enter_context(tc.tile_pool(name="ps_mm", bufs=2, space="PSUM"))

        # ================= S1: contiguous loads =================
        # q/k weights + hT first so S2 can start ASAP; everything else after.
        hT = s1pool.tile([128, 8 * S], F8)   # [c-part, kt*1024 + t]
        nc.sync.dma_start(hT[:, 0:4 * S], din["hT"][:, 0:4 * S])
        nc.scalar.dma_start(hT[:, 4 * S:], din["hT"][:, 4 * S:])
        w_sb = {}
        for i, name in enumerate(("wq", "wk", "wpk", "wpq", "wv")):
            w = s1pool.tile([128, 8 * 512], F8, tag=name)  # [k-part, kt*512 + c]
            eng = nc.sync if i % 2 == 0 else nc.scalar
            eng.dma_start(w[:], din[name][:, :])
            w_sb[name] = w
        relT_sb = s1pool.tile([128, 8 * S], F8)  # [k-part, kt*1024 + u]
        nc.scalar.dma_start(relT_sb[:, 0:4 * S], din["relT"][:, 0:4 * S])
        nc.scalar.dma_start(relT_sb[:, 4 * S:], din["relT"][:, 4 * S:])
        nc.scalar.dma_start(wo_sb[:], din["wo"][:, :])

        # v augmented matrix init (ones columns for the Z rows)
        nc.vector.memset(vaug[:], 0.0)
        nc.vector.memset(bass.AP(vaug[:].tensor, vaug[:].offset + 64,
                                 [[1024 * 8, 128], [1024, 8], [256, 4]]), 1.0)
        nc.vector.memset(bass.AP(vaug[:].tensor, vaug[:].offset + 128,
                                 [[1024 * 8, 128], [1024, 8], [256, 4]]), 1.0)

        # ---------------- per-chunk compute emitters ----------------
        def s3_ct(ct):
            # pos projections + clip extension for channel chunk ct
            for dst, wname, bias_ap in ((pkext, "wpk", None),
                                        (pqext, "wpq", pqb_sb)):
                for half in range(2):
                    ps = ps_mm.tile([128, 512], FP, tag="mm")
                    for kt in range(8):
                        nc.tensor.matmul(
                            ps[:],
                            w_sb[wname][:, 512 * kt + 128 * ct: 512 * kt + 128 * ct + 128],
                            relT_sb[:, S * kt + 512 * half: S * kt + 512 * half + 512],
                            start=(kt == 0), stop=(kt == 7),
                        )
                    o = W2 * ct + 512 + 512 * half
                    if bias_ap is None:
                        nc.scalar.copy(dst[:, o:o + 512], ps[:])
                    else:
                        nc.vector.tensor_scalar_add(
                            dst[:, o:o + 512], ps[:], bias_ap[:, ct:ct + 1])
                o = W2 * ct
                nc.vector.tensor_copy(
                    dst[:, o:o + 512],
                    dst[:, o + 512:o + 513].to_broadcast([128, 512]),
                )
                nc.vector.tensor_copy(
                    dst[:, o + 1536:o + 2048],
                    dst[:, o + 1535:o + 1536].to_broadcast([128, 512]),
                )

        def s2_ct(ct):
            # q/k in_proj for channel chunk ct (wq/qb pre-scaled by 1/SCALE)
            for half in range(2):
                psq = ps_mm.tile([128, 512], FP, tag="mm")
                psk = ps_mm.tile([128, 512], FP, tag="mm")
                for kt in range(8):
                    nc.tensor.matmul(
                        psq[:],
                        w_sb["wq"][:, 512 * kt + 128 * ct: 512 * kt + 128 * ct + 128],
                        hT[:, S * kt + 512 * half: S * kt + 512 * half + 512],
                        start=(kt == 0), stop=(kt == 7),
                    )
                for kt in range(8):
                    nc.tensor.matmul(
                        psk[:],
                        w_sb["wk"][:, 512 * kt + 128 * ct: 512 * kt + 128 * ct + 128],
                        hT[:, S * kt + 512 * half: S * kt + 512 * half + 512],
                        start=(kt == 0), stop=(kt == 7),
                    )
                nc.vector.tensor_scalar_add(
                    qT[:, S * ct + 512 * half: S * ct + 512 * half + 512],
                    psq[:], qb_sb[:, ct:ct + 1])
                nc.scalar.copy(
                    kT[:, S * ct + 512 * half: S * ct + 512 * half + 512],
                    psk[:])

        def s2_v(tt):
            # v in_proj for token block tt -> vaug (head-split + ones cols)
            psv = ps_mm.tile([128, 512], FP, tag="mm")
            for kt in range(8):
                nc.tensor.matmul(
                    psv[:],
                    hT[:, S * kt + 128 * tt: S * kt + 128 * tt + 128],
                    w_sb["wv"][:, 512 * kt: 512 * kt + 512],
                    start=(kt == 0), stop=(kt == 7),
                )
            base = vaug[:].offset + 1024 * tt
            nc.vector.scalar_tensor_tensor(
                bass.AP(vaug[:].tensor, base, [[1024 * 8, 128], [256, 4], [1, 64]]),
                bass.AP(psv[:].tensor, psv[:].offset, [[512, 128], [128, 4], [1, 64]]),
                1.0,
                bass.AP(vb_rep[:].tensor, vb_rep[:].offset, [[512, 128], [128, 4], [1, 64]]),
                op0=mybir.AluOpType.mult, op1=mybir.AluOpType.add,
            )
            nc.vector.scalar_tensor_tensor(
                bass.AP(vaug[:].tensor, base + 128 + 64, [[1024 * 8, 128], [256, 4], [1, 64]]),
                bass.AP(psv[:].tensor, psv[:].offset + 64, [[512, 128], [128, 4], [1, 64]]),
                1.0,
                bass.AP(vb_rep[:].tensor, vb_rep[:].offset + 64, [[512, 128], [128, 4], [1, 64]]),
                op0=mybir.AluOpType.mult, op1=mybir.AluOpType.add,
            )

        # ---------------- S4 machinery ----------------
        band_pool = ctx.enter_context(tc.tile_pool(name="band", bufs=6))
        gath_pool = ctx.enter_context(tc.tile_pool(name="gath", bufs=20))
        e1_pool = ctx.enter_context(tc.tile_pool(name="e1", bufs=12))
        misc_pool = ctx.enter_context(tc.tile_pool(name="misc", bufs=2))
        ps_s = ctx.enter_context(tc.tile_pool(name="ps_s", bufs=2, space="PSUM"))
        ps_ctx_pool = ctx.enter_context(tc.tile_pool(name="ps_ctx", bufs=1, space="PSUM"))

        def head_views(h):
            ct = h // 2
            po = 64 * (h % 2)
            return (
                qT[po:po + 64, S * ct: S * ct + S],
                kT[po:po + 64, S * ct: S * ct + S],
                pkext[po:po + 64, W2 * ct: W2 * ct + W2],
                pqext[po:po + 64, W2 * ct: W2 * ct + W2],
            )

        band2s = {}

        def get_band2(h):
            if h not in band2s:
                band2s[h] = dram.tile([S, 2 * BAND], F8, tag="band2",
                                      name=f"band2_{h}")
            return band2s[h]

        def produce_I(h, I):
            # one 128-row block of head h's banded tile: c2p (reversed) and
            # p2c halves; copies split across DVE/Act per chunk.
            if not mask & 8:
                return
            qT_h, kT_h, pk_h, pq_h = head_views(h)
            band2 = get_band2(h)
            bsb = band_pool.tile([128, 2 * BAND], F8, tag="band", name=f"cb{h}_{I}")
            for q, w in ((0, 512), (1, 512), (2, 128)):
                ps = ps_mm.tile([128, 512], FP, tag="mm", name=f"pc{h}_{I}_{q}")
                nc.tensor.matmul(
                    ps[:, :w],
                    qT_h[:, 128 * I: 128 * I + 128],
                    pk_h[:, 128 * I + 512 * q: 128 * I + 512 * q + w],
                    start=True, stop=True,
                )
                dst = bass.AP(bsb[:].tensor, bsb[:].offset + 1151 - 512 * q,
                              [[2 * BAND, 128], [-1, w]])
                if q == 1:
                    nc.scalar.copy(dst, ps[:, :w])
                else:
                    nc.vector.tensor_copy(dst, ps[:, :w])
            J = I
            m0 = 897 - 128 * J
            for q, w in ((0, 512), (1, 512), (2, 127)):
                ps = ps_mm.tile([128, 512], FP, tag="mm", name=f"pe{h}_{J}_{q}")
                nc.tensor.matmul(
                    ps[:, :w],
                    kT_h[:, 128 * J: 128 * J + 128],
                    pq_h[:, m0 + 512 * q: m0 + 512 * q + w],
                    start=True, stop=True,
                )
                dst = bsb[:, BAND + 512 * q: BAND + 512 * q + w]
                if q == 1:
                    nc.vector.tensor_copy(dst, ps[:, :w])
                else:
                    nc.scalar.copy(dst, ps[:, :w])
            nc.scalar.dma_start(band2[128 * I:128 * I + 128, :], bsb[:])

        def produce(h):
            get_band2(h)
            for I in range(NB):
                produce_I(h, I)

        def gather(h):
            gs = []
            if not mask & 16:
                return []
            band2 = band2s[h]
            for I in range(NB):
                g = gath_pool.tile([128, 2 * S], F8, tag="gath", name=f"g{h}_{I}")
                nc.sync.dma_start(
                    g[:].rearrange("p (a u) -> p a u", a=2),
                    bass.AP(band2[:].tensor, 128 * I * 2 * BAND + 127,
                            [[2 * BAND - 1, 128], [BAND, 2], [1, S]]),
                )
                gs.append(g)
            return gs

        def consume(h, gs, ph=None):
            qT_h, kT_h, pk_h, pq_h = head_views(h)
            ct = h // 2
            po = 64 * (h % 2)
            if not mask & 32:
                if ph is not None:
                    produce(ph)
                return
            ps_ctx = ps_ctx_pool.tile([128, S], FP, tag="ctx")
            e1s = []
            for J in range(NB):
                ps_sJ = ps_s.tile([128, S], FP, tag="s", name=f"s{h}_{J}")
                for c in range(2):
                    nc.tensor.matmul(
                        ps_sJ[:, 512 * c: 512 * c + 512],
                        kT_h[:, 128 * J: 128 * J + 128],
                        qT_h[:, 512 * c: 512 * c + 512],
                        start=True, stop=False,
                    )
                for I in range(NB):
                    nc.tensor.matmul(
                        ps_sJ[:, 128 * I: 128 * I + 128],
                        gs[I][:, 128 * J: 128 * J + 128],
                        id8[:],
                        start=False, stop=False,
                    )
                # p2c^T rows for this J-block via identity matmul accumulate
                for c in range(2):
                    nc.tensor.matmul(
                        ps_sJ[:, 512 * c: 512 * c + 512],
                        id8[:],
                        gs[J][:, S + 512 * c: S + 512 * c + 512],
                        start=False, stop=True,
                    )
                e1 = e1_pool.tile([128, S], F8, tag="e1", name=f"e1_{h}_{J}")
                nc.scalar.activation(e1[:], ps_sJ[:], EXPF)
                e1s.append(e1)
                # interleave one produce block of a later head: its PSUM
                # evacuation drains on DVE/Act while this head's next J-block
                # runs on PE.
                if ph is not None:
                    produce_I(ph, J)
            for J in range(NB):
                lhs = vaug[:, 1024 * J + 128 * h: 1024 * J + 128 * h + 128]
                for c in range(2):
                    nc.tensor.matmul(
                        ps_ctx[:, 512 * c: 512 * c + 512],
                        lhs,
                        e1s[J][:, 512 * c: 512 * c + 512],
                        start=(J == 0), stop=(J == 7),
                    )

            # stage ctx+Z rows out of PSUM immediately (frees ps_ctx for the
            # next head), then normalize via a small DRAM-broadcast round trip
            # that stays off the PSUM-recycle critical path.
            zrow = 64 if h % 2 == 0 else 0
            craw = misc_pool.tile([128, S], FP, tag="craw", name=f"cr{h}")
            nc.vector.tensor_copy(craw[po:po + 64, :], ps_ctx[po:po + 64, :])
            nc.vector.reciprocal(craw[zrow:zrow + 1, :], ps_ctx[zrow:zrow + 1, :])
            zdram = dram.tile([1, S], FP, tag="zdram", name=f"zd{h}")
            nc.sync.dma_start(zdram[:], craw[zrow:zrow + 1, :])
            rrep = misc_pool.tile([128, S], FP, tag="rrep", name=f"rr{h}")
            nc.sync.dma_start(
                rrep[po:po + 64, :],
                bass.AP(zdram[:].tensor, zdram[:].offset, [[0, 64], [1, S]]),
            )
            nc.vector.tensor_mul(
                ctxT[po:po + 64, S * ct: S * ct + S],
                craw[po:po + 64, :],
                rrep[po:po + 64, :],
            )

        # ---------------- emission schedule ----------------
        # Interleave S3/S2/v/produce so band DMA starts early; produce runs
        # 2-3 heads ahead of consume (blocks interleaved into consume's
        # J-loop), gather two heads ahead.
        if mask & 4:
            s2_ct(0)
        if mask & 2:
            s3_ct(0)
        produce(0)
        if mask & 4:
            s2_ct(1)
        if mask & 2:
            s3_ct(1)
        produce(1)
        if mask & 4:
            for tt in range(8):
                s2_v(tt)
        produce(2)
        gs_pend = {0: gather(0), 1: gather(1)}
        for h in range(8):
            ct = h + 2
            if ct < 4:
                if mask & 2:
                    s3_ct(ct)
                if mask & 4:
                    s2_ct(ct)
            if h + 2 < 8:
                gs_pend[h + 2] = gather(h + 2)
            consume(h, gs_pend.pop(h), ph=h + 3 if h + 3 < 8 else None)

        # ================= S5: output projection =================
        outp_pool = ctx.enter_context(tc.tile_pool(name="outp", bufs=2))
        ccins = [dram1.tile([512, HID], BF, tag=f"ccin{g}", name=f"ccin{g}") for g in range(2)]
        ccouts = [dram1.tile([256, HID], BF, tag=f"ccout{g}", name=f"ccout{g}") for g in range(2)]
        for g in range(2):
            tts = [2 * g, 2 * g + 1, 4 + 2 * g, 5 + 2 * g]
            for pos, tt in enumerate(tts if mask & 64 else []):
                hp = outp_pool.tile([128, HID], BF, tag="hp")
                for c in range(2):
                    ps = ps_mm.tile([128, 512], FP, tag="mm")
                    for ci in range(4):
                        nc.tensor.matmul(
                            ps[:],
                            ctxT[:, S * ci + 128 * tt: S * ci + 128 * tt + 128],
                            wo_sb[:, HID * ci + 512 * c: HID * ci + 512 * c + 512],
                            start=(ci == 0), stop=(ci == 3),
                        )
                    nc.vector.tensor_copy(hp[:, 512 * c: 512 * c + 512], ps[:])
                nc.sync.dma_start(ccins[g][128 * pos:128 * pos + 128, :], hp[:])
            if not mask & 128:
                pass
            elif sim_single_core:
                nc.sync.dma_start(
                    ccouts[g][:], ccins[g][256 * sim_rank: 256 * sim_rank + 256, :])
            else:
                nc.gpsimd.collective_compute(
                    "ReduceScatter", mybir.AluOpType.add,
                    replica_groups=[[0, 1], [2, 3], [4, 5], [6, 7]],
                    ins=[ccins[g].opt()], outs=[ccouts[g].opt()],
                )

        # ================= S7: residual + LayerNorm =================
        for tt in range(4 if mask & 256 else 0):
            g, pos = tt // 2, tt % 2
            ht = outp_pool.tile([128, HID], BF, tag="ln_h")
            nc.sync.dma_start(ht[:], ccouts[g][128 * pos:128 * pos + 128, :])
            hsum = outp_pool.tile([128, HID], FP, tag="ln_s")
            nc.sync.dma_start(hsum[:], din["hres"][128 * tt:128 * tt + 128, :])
            nc.vector.tensor_add(hsum[:], ht[:], hsum[:])

            stats = outp_pool.tile([128, 2, 6], FP, tag="bnst")
            for g2 in range(2):
                nc.vector.bn_stats(stats[:, g2, :], hsum[:, 512 * g2: 512 * g2 + 512])
            mv = outp_pool.tile([128, 2], FP, tag="bnmv")
            nc.vector.bn_aggr(mv[:], stats[:])
            rstd = outp_pool.tile([128, 1], FP, tag="rstd")
            nc.scalar.activation(rstd[:], mv[:, 1:2], SQRTF, bias=eps_sb[:])
            nc.vector.reciprocal(rstd[:], rstd[:])
            nc.vector.tensor_scalar(
                hsum[:], hsum[:], mv[:, 0:1], rstd[:],
                op0=mybir.AluOpType.subtract, op1=mybir.AluOpType.mult,
            )
            nc.vector.tensor_mul(hsum[:], hsum[:], lng_rep[:])
            nc.vector.tensor_add(hsum[:], hsum[:], lnb_rep[:])
            nc.sync.dma_start(dout[128 * tt:128 * tt + 128, :], hsum[:])


def _to_sbuf_blocks(a, nblk, blk, width):
    """[nblk*blk, width] -> [blk, nblk*width] : out[p, i*width+c] = a[i*blk+p, c]"""
    return np.ascontiguousarray(
        a.reshape(nblk, blk, width).transpose(1, 0, 2).reshape(blk, nblk * width))


def make_core_inputs(inputs):
    f8 = ml_dtypes.float8_e4m3
    hs = np.asarray(inputs["hidden_states"], np.float32)       # [4, S, HID]
    W = np.asarray(inputs["in_proj_w"], np.float32)            # [HID, 3*HID]
    rel = np.asarray(inputs["rel_embeddings"], np.float32)     # [S, HID]
    wpk_f = np.asarray(inputs["pos_proj_w"], np.float32)
    wpq_f = np.asarray(inputs["pos_q_proj_w"], np.float32)
    wo_f = np.asarray(inputs["out_w"], np.float32)
    qb_f = np.asarray(inputs["q_bias"], np.float32)
    vb_f = np.asarray(inputs["v_bias"], np.float32)
    pqb_f = np.asarray(inputs["pos_q_proj_b"], np.float32)
    ob_f = np.asarray(inputs["out_b"], np.float32)

    relT_l = _to_sbuf_blocks(rel.T.astype(f8), 8, 128, S)      # [128, 8*1024]
    inv_scale = np.float32(1.0 / SCALE)

    ins = []
    hT_cache = {}
    w_cache = {}
    for c in range(8):
        b, hg = c // 2, c % 2
        cs = slice(512 * hg, 512 * hg + 512)
        if b not in hT_cache:
            hT_cache[b] = _to_sbuf_blocks(hs[b].T.astype(f8), 8, 128, S)
        if hg not in w_cache:
            w_cache[hg] = {
                "wq": _to_sbuf_blocks((W[:, 0:1024][:, cs] * inv_scale).astype(f8), 8, 128, 512),
                "wk": _to_sbuf_blocks(W[:, 1024:2048][:, cs].astype(f8), 8, 128, 512),
                "wv": _to_sbuf_blocks(W[:, 2048:3072][:, cs].astype(f8), 8, 128, 512),
                "wpk": _to_sbuf_blocks(wpk_f[:, cs].astype(f8), 8, 128, 512),
                "wpq": _to_sbuf_blocks((wpq_f[:, cs] * inv_scale).astype(f8), 8, 128, 512),
                "wo": _to_sbuf_blocks(wo_f[cs, :].astype(f8), 4, 128, HID),
            }
        ins.append({
            "hT": hT_cache[b],
            "hres": hs[b, 512 * hg: 512 * hg + 512, :] + ob_f[None, :],
            "relT": relT_l,
            "qb": qb_f[cs] * inv_scale,
            "pqb": pqb_f[cs] * inv_scale,
            "vb": vb_f[cs],
            "lng": np.asarray(inputs["ln_g"], np.float32),
            "lnb": np.asarray(inputs["ln_b"], np.float32),
            **w_cache[hg],
        })
    return ins


_NC_CACHE = {}


def kernel(**inputs):
    from concourse.bass_utils import run_bass_kernel_spmd

    if "nc" not in _NC_CACHE:
        _NC_CACHE["nc"] = build_kernel()
    nc = _NC_CACHE["nc"]
    ins = make_core_inputs(inputs)
    res = run_bass_kernel_spmd(nc, ins, list(range(8)))
    out = np.zeros((4, S, HID), np.float32)
    for c in range(8):
        b, hg = c // 2, c % 2
        out[b, 512 * hg: 512 * hg + 512, :] = res.results[c]["out"]
    return out
